# revision 1
# baseline (speedup 1.0000x reference)
"""3-layer GAT (GATConv x3 + log_softmax) on 8 Trainium2 NeuronCores — V2.

Strategy (dst-sharded, edge-parallel within core), improvements over V1:
- fp16 node tables: L1/L2 rows [h(128) | a_src(4) | pad(4)] = 136 fp16 = 272B
  (vs 544B f32); L3 rows [h3(9) | a_src3(1) | pad(6)] = 16 fp16 = 32B with W3
  applied PRE-aggregation (out3 = sum alpha*(elu2@W3)[src], which commutes).
- Gathers read the AllGather output (Shared space) directly — no 27MB/layer
  staging copy.
- Whole-window edge phase: one score/mask/denominator/message/reduce op per
  128-dst window instead of per 16-slot chunk (5x fewer DVE instructions).
- Node transforms in fp16 (PE high-perf mode eligible).

Host-side prep identical to V1: per-core contiguous dst ranges, degree-desc
sort within core so each 128-node window has uniform max degree K; edges
grouped by (core, window, partition=dst slot, k=slot).
"""
import numpy as np

import concourse.bass as bass
import concourse.mybir as mybir
import concourse.tile as tile
from concourse.masks import make_identity

# ---- problem constants (hardcoded per contest rules) ----
N = 50000
E = 800000
F_IN = 300
HEADS = 4
PER_HEAD = 32
HID = 128
N_CLASSES = 9
NEG_SLOPE = 0.2

NC_ = 8
NPER = 6272          # nodes per core (49 * 128)
NPAD = NC_ * NPER    # 50176
P = 128
NWIN = NPER // P     # 49
DT = mybir.dt.float32
OOB = 1 << 20

f32 = mybir.dt.float32
f16 = mybir.dt.float16
AF = mybir.ActivationFunctionType

TW = 136   # L1/L2 table row width (fp16): h 128 | a_src 4 | pad 4
TW3 = 16   # L3 table row width (fp16): h3 9 | a_src3 1 | pad 6
STAGE_LOCAL = False  # gather from staged local copy instead of Shared AG buffer


# ----------------------------------------------------------------------------
# host-side graph prep (same as V1)
# ----------------------------------------------------------------------------
def prep_graph(edge_index):
    s = np.asarray(edge_index[0], dtype=np.int64)
    d = np.asarray(edge_index[1], dtype=np.int64)
    deg = np.bincount(d, minlength=N)

    old_of_new = np.full(NPAD, -1, dtype=np.int64)
    new_of_old = np.full(N, -1, dtype=np.int64)
    bounds = [min(c * NPER, N) for c in range(NC_ + 1)]
    for c in range(NC_):
        lo, hi = bounds[c], bounds[c + 1]
        nodes = np.arange(lo, hi)
        order = nodes[np.argsort(-deg[nodes], kind="stable")]
        old_of_new[c * NPER : c * NPER + len(order)] = order
        new_of_old[order] = c * NPER + np.arange(len(order))

    s_new = new_of_old[s]
    d_new = new_of_old[d]

    cores = []
    order_all = np.lexsort((s_new, d_new))
    s_sorted = s_new[order_all]
    d_sorted = d_new[order_all]
    starts = np.searchsorted(d_sorted, np.arange(NPAD))
    ends = np.searchsorted(d_sorted, np.arange(NPAD) + 1)

    for c in range(NC_):
        Ks = []
        idx_cols = []
        msk_cols = []
        for w in range(NWIN):
            base = c * NPER + w * P
            degs = ends[base : base + P] - starts[base : base + P]
            K = int(degs.max()) if len(degs) else 0
            Ks.append(K)
            if K == 0:
                continue
            iw = np.full((P, K), OOB, dtype=np.int32)
            mw = np.zeros((P, K), dtype=np.float16)
            for p in range(P):
                a, b = starts[base + p], ends[base + p]
                iw[p, : b - a] = s_sorted[a:b]
                mw[p, : b - a] = 1.0
            idx_cols.append(iw)
            msk_cols.append(mw)
        idx = (
            np.concatenate(idx_cols, axis=1)
            if idx_cols
            else np.zeros((P, 1), np.int32)
        )
        msk = (
            np.concatenate(msk_cols, axis=1)
            if msk_cols
            else np.zeros((P, 1), np.float16)
        )
        cores.append({"K": Ks, "idx": idx, "mask": msk})
    return cores, old_of_new, new_of_old


# ----------------------------------------------------------------------------
# walrus wait-split workaround (same as V1)
# ----------------------------------------------------------------------------
def split_excess_waits(nc, max_waits=1):
    import copy

    n = 0
    for f in nc.m.functions:
        for blk in f.blocks:
            new_insts = []
            for ins in blk.instructions:
                need = (
                    ins.sync_info is not None and len(ins.sync_info.on_wait) > max_waits
                ) or (
                    isinstance(ins, mybir.InstDMACopy)
                    and getattr(ins, "queue", "") == "qPoolDynamic"
                    and ins.sync_info is not None
                    and len(ins.sync_info.on_wait) > 0
                )
                if need:
                    for w in list(ins.sync_info.on_wait):
                        noop = mybir.InstNoOp(
                            name=f"wait_split_{n}",
                            text_hint="wait_split",
                            bass_nofuse=True,
                        )
                        n += 1
                        noop.engine = ins.engine
                        si = copy.deepcopy(ins.sync_info)
                        si.on_update = type(si.on_update)()
                        si.on_wait = type(si.on_wait)([copy.deepcopy(w)])
                        noop.sync_info = si
                        new_insts.append(noop)
                    ins.sync_info.on_wait = type(ins.sync_info.on_wait)()
                new_insts.append(ins)
            if n:
                blk.instructions = new_insts
    return n


# ----------------------------------------------------------------------------
# device kernel builder
# ----------------------------------------------------------------------------
def build_nc(Ks, S, repeat=1, no_gather=False, no_msg=False):
    """One SPMD program; per-core data differs only in tensor contents.

    repeat>1 re-runs the whole computation in one NEFF (idempotent) so
    device time can be measured as the marginal cost per repeat."""
    nc = bass.Bass()
    KMAX = max(Ks)

    xT = nc.declare_dram_parameter("xT", [F_IN, NPER], f16, isOutput=False)
    w1aug = nc.declare_dram_parameter("w1aug", [F_IN, TW], f16, isOutput=False)
    w2aug = nc.declare_dram_parameter("w2aug", [HID, TW], f16, isOutput=False)
    # L3 node transform: [W3 | W3@as3 | W3@ad3] -> 11 cols
    w3aug = nc.declare_dram_parameter("w3aug", [HID, 11], f16, isOutput=False)
    bias1 = nc.declare_dram_parameter("bias1", [P, HID], f32, isOutput=False)
    bias2 = nc.declare_dram_parameter("bias2", [P, HID], f32, isOutput=False)
    bias3 = nc.declare_dram_parameter("bias3", [P, N_CLASSES], f32, isOutput=False)
    idx_in = nc.declare_dram_parameter("idx", [P, S], mybir.dt.int32, isOutput=False)
    mask_in = nc.declare_dram_parameter("mask", [P, S], f16, isOutput=False)
    out_ext = nc.declare_dram_parameter("out", [NPER, N_CLASSES], f32, isOutput=True)

    shard = [
        nc.dram_tensor(f"shard{l}", [NPER, TW if l < 2 else TW3], f16)
        for l in range(3)
    ]
    table_sh = [
        nc.dram_tensor(
            f"table_sh{l}", [NPAD, TW if l < 2 else TW3], f16, addr_space="Shared"
        )
        for l in range(3)
    ]
    if STAGE_LOCAL:
        table_loc = [
            nc.dram_tensor(f"table_loc{l}", [NPAD, TW if l < 2 else TW3], f16)
            for l in range(3)
        ]

    rg = [list(range(NC_))]

    with tile.TileContext(nc) as tc:
        with (
            tc.tile_pool(name="resident", bufs=1) as rp,
            tc.tile_pool(name="sbuf", bufs=3) as pool,
            tc.tile_pool(name="gp", bufs=3) as gpool,
            tc.tile_pool(name="mp", bufs=2) as mpool,
            tc.tile_pool(name="aggp", bufs=2) as agpool,
            tc.tile_pool(name="psum", bufs=2, space="PSUM") as pp,
            tc.tile_pool(name="psum_t", bufs=2, space="PSUM") as ppt,
        ):
            # ---------- residents ----------
            idx_sb = rp.tile([P, S], mybir.dt.int32)
            nc.gpsimd.dma_start(out=idx_sb[:], in_=idx_in[:])
            mask_sb = rp.tile([P, S], f16)
            nc.sync.dma_start(out=mask_sb[:], in_=mask_in[:])
            ident = rp.tile([P, P], f32)
            make_identity(nc, ident[:])
            b1_sb = rp.tile([P, HID], f32)
            nc.sync.dma_start(out=b1_sb[:], in_=bias1[:])
            b2_sb = rp.tile([P, HID], f32)
            nc.sync.dma_start(out=b2_sb[:], in_=bias2[:])
            b3_sb = rp.tile([P, N_CLASSES], f32)
            nc.sync.dma_start(out=b3_sb[:], in_=bias3[:])
            w2aug_sb = rp.tile([P, TW], f16)
            nc.sync.dma_start(out=w2aug_sb[:], in_=w2aug[:])
            w3aug_sb = rp.tile([P, 11], f16)
            nc.sync.dma_start(out=w3aug_sb[:], in_=w3aug[:])
            w1_sb = rp.tile([P, 3 * TW], f16)  # 3 k-chunks of w1aug
            for kc in range(3):
                kd = min(P, F_IN - kc * P)
                nc.sync.dma_start(
                    out=w1_sb[:kd, kc * TW : kc * TW + TW],
                    in_=w1aug[kc * P : kc * P + kd, :],
                )
            # per-layer a_dst of own nodes [P, NWIN*4] (L3 uses 1 col/window)
            adst = [rp.tile([P, NWIN * 4], f32, name=f"adst{l}") for l in range(3)]

            bound = nc.gpsimd.to_reg(NPAD - 1)

            # memset gather pool slots once (avoid NaN garbage in pad slots)
            gz = [
                gpool.tile([P, KMAX, TW], f16, tag="g", name=f"gz{i}")
                for i in range(3)
            ]
            for t in gz:
                nc.vector.memset(t[:], 0.0)
            gz3 = [
                gpool.tile([P, KMAX, TW3], f16, tag="g3", name=f"gz3_{i}")
                for i in range(3)
            ]
            for t in gz3:
                nc.vector.memset(t[:], 0.0)

            for _rep in range(repeat):
                # ---------- layer-1 node transform ----------
                # shard0 rows = [h1 | a_src1 | a_dst1-pad], h1 = x @ W1 etc.
                for t in range(NWIN):
                    hpsum = pp.tile([P, TW], f32, tag="hpsum")
                    for kc in range(3):
                        kd = min(P, F_IN - kc * P)
                        xt = pool.tile([P, P], f16, tag="xt")
                        nc.sync.dma_start(
                            out=xt[:kd, :],
                            in_=xT[kc * P : kc * P + kd, t * P : (t + 1) * P],
                        )
                        nc.tensor.matmul(
                            out=hpsum[:],
                            lhsT=xt[:kd, :],
                            rhs=w1_sb[:kd, kc * TW : kc * TW + TW],
                            start=(kc == 0),
                            stop=(kc == 2),
                        )
                    hrow = pool.tile([P, TW], f16, tag="hrow")
                    nc.vector.tensor_copy(out=hrow[:], in_=hpsum[:])
                    nc.vector.tensor_copy(
                        out=adst[0][:, t * 4 : (t + 1) * 4], in_=hpsum[:, 132:136]
                    )
                    nc.sync.dma_start(out=shard[0][t * P : (t + 1) * P, :], in_=hrow[:])

                # ---------- per-layer: allgather + edge phase ----------
                for l in range(3):
                    nc.gpsimd.collective_compute(
                        "AllGather",
                        mybir.AluOpType.bypass,
                        ins=[shard[l][:]],
                        outs=[table_sh[l][:]],
                        replica_groups=rg,
                    )
                    if STAGE_LOCAL:
                        nc.sync.dma_start(out=table_loc[l][:], in_=table_sh[l][:])
                    gather_src = table_loc[l] if STAGE_LOCAL else table_sh[l]
                    heads = 4 if l < 2 else 1
                    tw_l = TW if l < 2 else TW3
                    gtag = "g" if l < 2 else "g3"
                    off = 0
                    GW = 7  # windows per batched-tail group (NWIN = 7*7)
                    agg_g = None
                    for w in range(NWIN):
                        gi = w % GW
                        if gi == 0:
                            agg_g = agpool.tile(
                                [P, GW, HID if l < 2 else N_CLASSES],
                                f32,
                                tag="agg_g" if l < 2 else "agg3_g",
                            )
                        K = Ks[w]
                        g = gpool.tile([P, KMAX, tw_l], f16, tag=gtag)
                        for k in range(K):
                            nc.gpsimd.indirect_dma_start(
                                out=g[:, k, :],
                                out_offset=None,
                                in_=gather_src[:, :],
                                in_offset=bass.IndirectOffsetOnAxis(
                                    ap=idx_sb[:, off + k : off + k + 1],
                                    axis=0,
                                ),
                                bounds_check=bound,
                                oob_is_err=False,
                            )
                        # ---- scores ----
                        if l < 2:
                            lr = pool.tile([P, KMAX, 4], f32, tag="lr")
                            nc.vector.tensor_tensor(
                                out=lr[:, :K, :],
                                in0=g[:, :K, HID : HID + 4],
                                in1=adst[l][:, w * 4 : (w + 1) * 4]
                                .unsqueeze(1)
                                .to_broadcast([P, K, 4]),
                                op=mybir.AluOpType.add,
                            )
                            nc.vector.scalar_tensor_tensor(
                                out=lr[:, :K, :],
                                in0=lr[:, :K, :],
                                scalar=NEG_SLOPE,
                                in1=lr[:, :K, :],
                                op0=mybir.AluOpType.mult,
                                op1=mybir.AluOpType.max,
                            )
                            e = pool.tile([P, KMAX, 4], f32, tag="e")
                            nc.scalar.activation(
                                out=e[:, :K, :], in_=lr[:, :K, :], func=AF.Exp
                            )
                            nc.vector.tensor_tensor(
                                out=e[:, :K, :],
                                in0=e[:, :K, :],
                                in1=mask_sb[:, off : off + K]
                                .unsqueeze(2)
                                .to_broadcast([P, K, 4]),
                                op=mybir.AluOpType.mult,
                            )
                            den = pool.tile([P, 4], f32, tag="den")
                            nc.vector.tensor_reduce(
                                out=den[:, :],
                                in_=e[:, :K, :].transpose([0, 2, 1]),
                                axis=mybir.AxisListType.X,
                                op=mybir.AluOpType.add,
                            )
                            nc.vector.tensor_scalar_add(
                                out=den[:, :], in0=den[:, :], scalar1=1e-30
                            )
                            rden = pool.tile([P, 4], f32, tag="rden")
                            nc.vector.reciprocal(out=rden[:, :], in_=den[:, :])
                            # alpha = e * rden[dst]  (normalization folded in)
                            alpha = pool.tile([P, KMAX, 4], f32, tag="alpha")
                            nc.vector.tensor_tensor(
                                out=alpha[:, :K, :],
                                in0=e[:, :K, :],
                                in1=rden[:, :].unsqueeze(1).to_broadcast([P, K, 4]),
                                op=mybir.AluOpType.mult,
                            )
                            # ---- messages: m[p,h,c,k] so the k-reduce is contiguous
                            m = mpool.tile([P, 4, PER_HEAD, KMAX], f16, tag="m")
                            if no_msg:
                                nc.vector.memset(m[:, 0, 0, :], 0.0)
                                nc.vector.memset(agg_g[:, gi, :], 0.0)
                            else:
                                nc.vector.tensor_tensor(
                                    out=m[:, :, :, :K].rearrange(
                                        "p h c k -> p k h c"
                                    ),
                                    in0=g[:, :K, 0:HID].rearrange(
                                        "p k (h c) -> p k h c", h=4
                                    ),
                                    in1=alpha[:, :K, :].unsqueeze(3).to_broadcast(
                                        [P, K, 4, PER_HEAD]
                                    ),
                                    op=mybir.AluOpType.mult,
                                )
                                nc.vector.tensor_reduce(
                                    out=agg_g[:, gi, :],
                                    in_=m[:, :, :, :K],
                                    axis=mybir.AxisListType.X,
                                    op=mybir.AluOpType.add,
                                )
                        else:
                            lr = pool.tile([P, KMAX, 1], f32, tag="lr3")
                            nc.vector.tensor_tensor(
                                out=lr[:, :K, :],
                                in0=g[:, :K, 9:10],
                                in1=adst[2][:, w * 4 : w * 4 + 1]
                                .unsqueeze(1)
                                .to_broadcast([P, K, 1]),
                                op=mybir.AluOpType.add,
                            )
                            nc.vector.scalar_tensor_tensor(
                                out=lr[:, :K, :],
                                in0=lr[:, :K, :],
                                scalar=NEG_SLOPE,
                                in1=lr[:, :K, :],
                                op0=mybir.AluOpType.mult,
                                op1=mybir.AluOpType.max,
                            )
                            e = pool.tile([P, KMAX, 1], f32, tag="e3")
                            nc.scalar.activation(
                                out=e[:, :K, :], in_=lr[:, :K, :], func=AF.Exp
                            )
                            nc.vector.tensor_tensor(
                                out=e[:, :K, 0],
                                in0=e[:, :K, 0],
                                in1=mask_sb[:, off : off + K],
                                op=mybir.AluOpType.mult,
                            )
                            den = pool.tile([P, 1], f32, tag="den3")
                            nc.vector.tensor_reduce(
                                out=den[:, :],
                                in_=e[:, :K, 0],
                                axis=mybir.AxisListType.X,
                                op=mybir.AluOpType.add,
                            )
                            nc.vector.tensor_scalar_add(
                                out=den[:, :], in0=den[:, :], scalar1=1e-30
                            )
                            rden = pool.tile([P, 1], f32, tag="rden3")
                            nc.vector.reciprocal(out=rden[:, :], in_=den[:, :])
                            m = pool.tile([P, KMAX, N_CLASSES], f32, tag="m3")
                            nc.vector.tensor_tensor(
                                out=m[:, :K, :],
                                in0=g[:, :K, 0:N_CLASSES],
                                in1=e[:, :K, :].to_broadcast([P, K, N_CLASSES]),
                                op=mybir.AluOpType.mult,
                            )
                            out_raw = pool.tile([P, N_CLASSES], f32, tag="oraw3")
                            nc.vector.tensor_reduce(
                                out=out_raw[:],
                                in_=m[:, :K, :].transpose([0, 2, 1]),
                                axis=mybir.AxisListType.X,
                                op=mybir.AluOpType.add,
                            )
                            nc.vector.tensor_tensor(
                                out=agg_g[:, gi, :],
                                in0=out_raw[:],
                                in1=rden[:, 0:1].to_broadcast([P, N_CLASSES]),
                                op=mybir.AluOpType.mult,
                            )
                        off += K

                        # ---- batched tail once per group of GW windows ----
                        if gi == GW - 1:
                            w0 = w - (GW - 1)
                            if l < 2:
                                bsb = b1_sb if l == 0 else b2_sb
                                y = pool.tile([P, GW, HID], f32, tag="y_g")
                                nc.vector.tensor_tensor(
                                    out=y[:],
                                    in0=agg_g[:],
                                    in1=bsb[:].unsqueeze(1).to_broadcast(
                                        [P, GW, HID]
                                    ),
                                    op=mybir.AluOpType.add,
                                )
                                neg = pool.tile([P, GW, HID], f32, tag="neg_g")
                                nc.vector.tensor_scalar_min(
                                    out=neg[:], in0=y[:], scalar1=0.0
                                )
                                en = pool.tile([P, GW, HID], f32, tag="en_g")
                                nc.scalar.activation(
                                    out=en[:], in_=neg[:], func=AF.Exp
                                )
                                elu = pool.tile([P, GW, HID], f32, tag="elu_g")
                                nc.vector.scalar_tensor_tensor(
                                    out=elu[:],
                                    in0=y[:],
                                    scalar=0.0,
                                    in1=en[:],
                                    op0=mybir.AluOpType.max,
                                    op1=mybir.AluOpType.add,
                                )
                                nc.vector.tensor_scalar_add(
                                    out=elu[:], in0=elu[:], scalar1=-1.0
                                )
                                for i in range(GW):
                                    ww = w0 + i
                                    eluT_p = ppt.tile([P, P], f32, tag="eluT_p")
                                    nc.tensor.transpose(
                                        out=eluT_p[:],
                                        in_=elu[:, i, :],
                                        identity=ident[:],
                                    )
                                    eluT = pool.tile([P, P], f16, tag="eluT")
                                    nc.vector.tensor_copy(
                                        out=eluT[:], in_=eluT_p[:]
                                    )
                                    if l == 0:
                                        h2psum = pp.tile([P, TW], f32, tag="hpsum")
                                        nc.tensor.matmul(
                                            out=h2psum[:],
                                            lhsT=eluT[:],
                                            rhs=w2aug_sb[:],
                                            start=True,
                                            stop=True,
                                        )
                                        srow = pool.tile([P, TW], f16, tag="srow")
                                        nc.vector.tensor_copy(
                                            out=srow[:], in_=h2psum[:]
                                        )
                                        nc.vector.tensor_copy(
                                            out=adst[1][:, ww * 4 : ww * 4 + 4],
                                            in_=h2psum[:, 132:136],
                                        )
                                        nc.sync.dma_start(
                                            out=shard[1][ww * P : (ww + 1) * P, :],
                                            in_=srow[:],
                                        )
                                    else:
                                        napsum = ppt.tile([P, 11], f32, tag="napsum")
                                        nc.tensor.matmul(
                                            out=napsum[:],
                                            lhsT=eluT[:],
                                            rhs=w3aug_sb[:, :11],
                                            start=True,
                                            stop=True,
                                        )
                                        srow = pool.tile([P, TW3], f16, tag="srow3")
                                        nc.vector.memset(srow[:], 0.0)
                                        nc.vector.tensor_copy(
                                            out=srow[:, 0:10], in_=napsum[:, 0:10]
                                        )
                                        nc.vector.tensor_copy(
                                            out=adst[2][:, ww * 4 : ww * 4 + 1],
                                            in_=napsum[:, 10:11],
                                        )
                                        nc.sync.dma_start(
                                            out=shard[2][ww * P : (ww + 1) * P, :],
                                            in_=srow[:],
                                        )
                            else:
                                # batched bias+elu+log_softmax over GW windows
                                y0 = pool.tile([P, GW, N_CLASSES], f32, tag="y90g")
                                nc.vector.tensor_tensor(
                                    out=y0[:],
                                    in0=agg_g[:],
                                    in1=b3_sb[:].unsqueeze(1).to_broadcast(
                                        [P, GW, N_CLASSES]
                                    ),
                                    op=mybir.AluOpType.add,
                                )
                                n9 = pool.tile([P, GW, N_CLASSES], f32, tag="n9g")
                                nc.vector.tensor_scalar_min(
                                    out=n9[:], in0=y0[:], scalar1=0.0
                                )
                                nc.scalar.activation(out=n9[:], in_=n9[:], func=AF.Exp)
                                yb = pool.tile([P, GW, N_CLASSES], f32, tag="y9g")
                                nc.vector.scalar_tensor_tensor(
                                    out=yb[:],
                                    in0=y0[:],
                                    scalar=0.0,
                                    in1=n9[:],
                                    op0=mybir.AluOpType.max,
                                    op1=mybir.AluOpType.add,
                                )
                                nc.vector.tensor_scalar_add(
                                    out=yb[:], in0=yb[:], scalar1=-1.0
                                )
                                e9 = pool.tile([P, GW, N_CLASSES], f32, tag="e9g")
                                nc.scalar.activation(out=e9[:], in_=yb[:], func=AF.Exp)
                                s9 = pool.tile([P, GW], f32, tag="s9g")
                                nc.vector.reduce_sum(
                                    out=s9[:], in_=e9[:], axis=mybir.AxisListType.X
                                )
                                l9 = pool.tile([P, GW], f32, tag="l9g")
                                nc.scalar.activation(out=l9[:], in_=s9[:], func=AF.Ln)
                                o9 = pool.tile([P, GW, N_CLASSES], f32, tag="o9g")
                                nc.vector.tensor_tensor(
                                    out=o9[:],
                                    in0=yb[:],
                                    in1=l9[:].unsqueeze(2).to_broadcast(
                                        [P, GW, N_CLASSES]
                                    ),
                                    op=mybir.AluOpType.subtract,
                                )
                                for i in range(GW):
                                    ww = w0 + i
                                    nc.sync.dma_start(
                                        out=out_ext[ww * P : (ww + 1) * P, :],
                                        in_=o9[:, i, :],
                                    )

    return nc


# ----------------------------------------------------------------------------
# host wrapper
# ----------------------------------------------------------------------------
def _np(x):
    return np.asarray(x)


def make_in_maps(inputs):
    x = _np(inputs["x"]).astype(np.float32)
    edge_index = _np(inputs["edge_index"])
    W1 = _np(inputs["W1"]).astype(np.float32)
    as1 = _np(inputs["as1"]).astype(np.float32)
    ad1 = _np(inputs["ad1"]).astype(np.float32)
    b1 = _np(inputs["b1"]).astype(np.float32)
    W2 = _np(inputs["W2"]).astype(np.float32)
    as2 = _np(inputs["as2"]).astype(np.float32)
    ad2 = _np(inputs["ad2"]).astype(np.float32)
    b2 = _np(inputs["b2"]).astype(np.float32)
    W3 = _np(inputs["W3"]).astype(np.float32)
    as3 = _np(inputs["as3"]).astype(np.float32)
    ad3 = _np(inputs["ad3"]).astype(np.float32)
    b3 = _np(inputs["b3"]).astype(np.float32)

    cores, old_of_new, new_of_old = prep_graph(edge_index)

    Ks = [max(cores[c]["K"][w] for c in range(NC_)) for w in range(NWIN)]
    S = sum(Ks)

    idx_u = np.full((NC_, P, S), OOB, dtype=np.int32)
    msk_u = np.zeros((NC_, P, S), dtype=np.float16)
    for c in range(NC_):
        off_u = 0
        off_c = 0
        for w in range(NWIN):
            Kc = cores[c]["K"][w]
            Ku = Ks[w]
            if Kc > 0:
                idx_u[c, :, off_u : off_u + Kc] = cores[c]["idx"][:, off_c : off_c + Kc]
                msk_u[c, :, off_u : off_u + Kc] = cores[c]["mask"][:, off_c : off_c + Kc]
            off_u += Ku
            off_c += Kc

    def blockdiag(a):  # [H, C] -> [H*C, H]
        H, C = a.shape
        out = np.zeros((H * C, H), np.float32)
        for h in range(H):
            out[h * C : (h + 1) * C, h] = a[h]
        return out

    as1b, ad1b = blockdiag(as1), blockdiag(ad1)
    as2b, ad2b = blockdiag(as2), blockdiag(ad2)
    w1aug = np.concatenate([W1, W1 @ as1b, W1 @ ad1b], axis=1).astype(np.float16)
    w2aug = np.concatenate([W2, W2 @ as2b, W2 @ ad2b], axis=1).astype(np.float16)
    w3aug = np.concatenate(
        [W3, (W3 @ as3[0])[:, None], (W3 @ ad3[0])[:, None]], axis=1
    ).astype(np.float16)

    xT = np.zeros((F_IN, NPAD), np.float16)
    real = old_of_new >= 0
    xT[:, real] = x[old_of_new[real]].T.astype(np.float16)

    b1_bc = np.broadcast_to(b1, (P, HID)).copy()
    b2_bc = np.broadcast_to(b2, (P, HID)).copy()
    b3_bc = np.broadcast_to(b3, (P, N_CLASSES)).copy()

    in_maps = []
    for c in range(NC_):
        in_maps.append(
            {
                "xT": np.ascontiguousarray(xT[:, c * NPER : (c + 1) * NPER]),
                "w1aug": w1aug,
                "w2aug": w2aug,
                "w3aug": w3aug,
                "bias1": b1_bc,
                "bias2": b2_bc,
                "bias3": b3_bc,
                "idx": idx_u[c],
                "mask": msk_u[c],
            }
        )
    return in_maps, Ks, S, old_of_new


def kernel(**inputs):
    from concourse.bass_utils import run_bass_kernel_spmd

    in_maps, Ks, S, old_of_new = make_in_maps(inputs)
    nc = build_nc(Ks, S)
    split_excess_waits(nc)
    res = run_bass_kernel_spmd(nc, in_maps, list(range(NC_)))
    out = np.zeros((N, N_CLASSES), np.float32)
    for c in range(NC_):
        rows = old_of_new[c * NPER : (c + 1) * NPER]
        m = rows >= 0
        out[rows[m]] = res.results[c]["out"][m]
    return out

